# revision 9
# baseline (speedup 1.0000x reference)
"""Trainium2 Bass kernel: per-point 3x3 Gaussian covariance from quaternion + log_scale.

cov = R diag(exp(log_scale)) R^T with R from the normalized quaternion.

v3 design: grouped [P, 3f] fp16 tiles with shifted views; normalization is
deferred (raw quaternion products; s/2 = 1/|q|^2 folded into the per-column
scales Esc_j = exp(ls_j/2)/|q|^2) so VectorE never stalls on the Ln/Exp
chain. The diagonal rotation entries come from the same squares used for
|q|^2 (u-trick):
  rdr = (ww+xx-yy-zz, ww-xx+yy-zz, ww-xx-yy+zz)   [= 2*n2*R_ii]
  rm  = (xy-wz, yz-wx, xz-wy)                     [= n2*R_off/...]
  M rows grouped T0=(M00,M11,M22) T1=(M01,M12,M20) T2=(M02,M10,M21);
  Gram = squares (ScalarE!) + rotated crosses + slot-aligned adds.
cov22 is reconstructed on the host from trace(cov) = sum(exp(log_scale)).
Output: 5 unique entries planar fp16 [5, NPC]; host assembles [N,3,3] fp32.
"""

import os
import numpy as np

import concourse.bass as bass
import concourse.bacc as bacc
import concourse.mybir as mybir
from concourse.tile import TileContext
from concourse.bass_utils import run_bass_kernel_spmd

AF = mybir.ActivationFunctionType
ALU = mybir.AluOpType
FP32 = mybir.dt.float32
F16 = mybir.dt.float16

N_CORES = 8
N_FULL = 4_000_000
P = 128
R = 3908
NPC = P * R
F = int(os.environ.get("KERNEL_F", "672"))
SQ_ENG = os.environ.get("KERNEL_SQ_ENG", "s")   # gram squares: s(calar) or v(ector)

_built = {}


def _build():
    key = (F, SQ_ENG)
    if key in _built:
        return _built[key]

    nc = bacc.Bacc("TRN2", target_bir_lowering=False, debug=False, num_devices=N_CORES)
    q = nc.dram_tensor("q", [NPC, 4], FP32, kind="ExternalInput")
    ls = nc.dram_tensor("ls", [NPC, 3], FP32, kind="ExternalInput")
    cov5 = nc.dram_tensor("cov5", [5, NPC], F16, kind="ExternalOutput")

    qv = q.ap().rearrange("(p r) c -> p (r c)", p=P)
    lsv = ls.ap().rearrange("(p r) c -> p (r c)", p=P)
    ov = cov5.ap().rearrange("e (p r) -> p e r", p=P)    # [128, 5, R]

    with TileContext(nc) as tc:
        with (
            tc.tile_pool(name="io", bufs=2) as io,
            tc.tile_pool(name="wk", bufs=1) as wk,
        ):
            t0 = 0
            while t0 < R:
                f = min(F, R - t0)
                _tile_body(nc, io, wk, qv, lsv, ov, t0, f)
                t0 += f

    nc.compile()
    _built[key] = nc
    return nc


def _tile_body(nc, io, wk, qv, lsv, ov, t0, f):
    def t2(n_f, dt, tag):
        return io.tile([P, n_f * f], dt, tag=tag, name=f"{tag}_{t0}")

    def t1(n_f, tag, dt=F16):
        return wk.tile([P, n_f * f], dt, tag=tag, name=f"{tag}_{t0}")

    qt = t2(4, FP32, "qt")
    lst = t2(3, FP32, "lst")
    nc.sync.dma_start(out=qt, in_=qv[:, 4 * t0:4 * (t0 + f)])
    nc.sync.dma_start(out=lst, in_=lsv[:, 3 * t0:3 * (t0 + f)])
    qc = qt.rearrange("p (f c) -> p f c", c=4)
    lsc = lst.rearrange("p (f c) -> p f c", c=3)

    # ---------------- ScalarE, phase 1: deinterleave + exps ----------------
    Q4 = t2(4, F16, "Q4")                                  # [w|x|y|z]
    for c in range(4):
        nc.scalar.copy(out=Q4[:, c * f:(c + 1) * f], in_=qc[:, :, c])
    ER3 = t2(3, F16, "ER3")                                # [e0|e1|e2]
    for j in range(3):
        nc.scalar.activation(ER3[:, j * f:(j + 1) * f], lsc[:, :, j], AF.Exp,
                             scale=0.5)

    # ---------------- VectorE: squares -> n2, diag entries -----------------
    q2 = t1(4, "q2")                                       # ww|xx|yy|zz
    nc.vector.tensor_mul(q2, Q4, Q4)
    u1 = t1(1, "u1"); u2 = t1(1, "u2"); u3 = t1(1, "u3"); u4 = t1(1, "u4")
    nc.vector.tensor_add(u1, q2[:, :f], q2[:, f:2 * f])            # ww+xx
    nc.vector.tensor_add(u2, q2[:, 2 * f:3 * f], q2[:, 3 * f:])    # yy+zz
    n2 = t2(1, F16, "n2")
    nc.vector.tensor_add(n2, u1, u2)
    nc.vector.tensor_sub(u3, q2[:, :f], q2[:, f:2 * f])            # ww-xx
    nc.vector.tensor_sub(u4, q2[:, 2 * f:3 * f], q2[:, 3 * f:])    # yy-zz
    rdr = t1(3, "rdr")                                     # 2*n2*(r00,r11,r22)
    nc.vector.tensor_sub(rdr[:, :f], u1, u2)
    nc.vector.tensor_add(rdr[:, f:2 * f], u3, u4)
    nc.vector.tensor_sub(rdr[:, 2 * f:], u3, u4)

    # ---------------- ScalarE, phase 2: 1/n2 + Q4b permute -----------------
    L = t2(1, F16, "L")
    nc.scalar.activation(L, n2, AF.Ln)
    sh = t2(1, F16, "sh")                                  # 1/n2
    nc.scalar.activation(sh, L, AF.Exp, scale=-1.0)
    Q4b = t2(3, F16, "Q4b")                                # [z|x|y]
    nc.scalar.copy(out=Q4b[:, :f], in_=Q4[:, 3 * f:])
    nc.scalar.copy(out=Q4b[:, f:], in_=Q4[:, f:3 * f])

    def bcast3(ap_1f):
        return ap_1f.rearrange("p (o f) -> p o f", o=1).to_broadcast([P, 3, f])

    # ---------------- VectorE: off-diagonal raw products -------------------
    Um = t1(3, "Um")                                       # (xy, yz, xz)
    nc.vector.tensor_mul(Um[:, :2 * f], Q4[:, f:3 * f], Q4[:, 2 * f:])
    nc.vector.tensor_mul(Um[:, 2 * f:], Q4[:, f:2 * f], Q4[:, 3 * f:])
    Vm = t1(3, "Vm")                                       # (wz, wx, wy)
    nc.vector.tensor_mul(Vm.rearrange("p (o f) -> p o f", o=3),
                         bcast3(Q4[:, :f]),
                         Q4b.rearrange("p (o f) -> p o f", o=3))
    rm = t1(3, "rm")                                       # (xy-wz, yz-wx, xz-wy)
    nc.vector.tensor_sub(rm, Um, Vm)
    padd4 = t1(4, "padd4")                                 # [p02|p10|p21|p02]
    nc.vector.tensor_add(padd4[:, f:], Um, Vm)             # (p10,p21,p02)
    nc.vector.tensor_copy(out=padd4[:, :f], in_=padd4[:, 3 * f:])

    # ---------------- column scales ----------------------------------------
    # rdr = n2*(R00,R11,R22); rm/padd = n2*R_ij/2 for off entries.
    # diag col scale: e_j/n2 ; off col scale: 2*e_j/n2
    Escd = t1(3, "Escd")
    nc.vector.tensor_mul(Escd.rearrange("p (o f) -> p o f", o=3),
                         ER3.rearrange("p (o f) -> p o f", o=3),
                         bcast3(sh))                       # e_j/n2
    Esco5 = t1(5, "Esco5")
    nc.vector.tensor_scalar(Esco5[:, :3 * f], Escd, 2.0, None, ALU.mult)
    nc.vector.tensor_copy(out=Esco5[:, 3 * f:], in_=Esco5[:, :2 * f])

    # ---------------- M rows ----------------------------------------------
    T0e = t2(4, F16, "T0e"); T1e = t2(4, F16, "T1e"); T2e = t2(4, F16, "T2e")
    nc.vector.tensor_mul(T0e[:, :3 * f], rdr, Escd)        # (M00,M11,M22)
    nc.vector.tensor_mul(T1e[:, :3 * f], rm, Esco5[:, f:4 * f])    # (M01,M12,M20)
    nc.vector.tensor_mul(T2e[:, :3 * f], padd4[:, :3 * f], Esco5[:, 2 * f:])  # (M02,M10,M21)
    for Te in (T0e, T1e, T2e):
        nc.scalar.copy(out=Te[:, 3 * f:], in_=Te[:, :f])

    # ---------------- Gram -------------------------------------------------
    S0 = t2(2, F16, "S0"); S1 = t2(2, F16, "S1"); S2 = t2(2, F16, "S2")
    if SQ_ENG == "s":
        nc.scalar.activation(S0, T0e[:, :2 * f], AF.Square)
        nc.scalar.activation(S1, T1e[:, :2 * f], AF.Square)
        nc.scalar.activation(S2, T2e[:, :2 * f], AF.Square)
    else:
        nc.vector.tensor_mul(S0, T0e[:, :2 * f], T0e[:, :2 * f])
        nc.vector.tensor_mul(S1, T1e[:, :2 * f], T1e[:, :2 * f])
        nc.vector.tensor_mul(S2, T2e[:, :2 * f], T2e[:, :2 * f])
    X = t1(3, "X"); Y = t1(3, "Y"); Z = t1(3, "Z")
    nc.vector.tensor_mul(X, T0e[:, :3 * f], T2e[:, f:])
    nc.vector.tensor_mul(Y, T1e[:, :3 * f], T0e[:, f:])
    nc.vector.tensor_mul(Z, T2e[:, :3 * f], T1e[:, f:])

    ot = t2(5, F16, "ot")                 # [c00|c11|c01|c12|c02]
    dsum = t1(2, "dsum"); osum = t1(3, "osum")
    nc.vector.tensor_add(dsum, S0, S1)
    nc.vector.tensor_add(ot[:, :2 * f], dsum, S2)
    nc.vector.tensor_add(osum, X, Y)
    nc.vector.tensor_add(ot[:, 2 * f:], osum, Z)

    otv = ot.rearrange("p (e f) -> p e f", e=5)
    nc.sync.dma_start(out=ov[:, :, t0:t0 + f], in_=otv)


def _pad_and_shard(quaternion, log_scale):
    n = quaternion.shape[0]
    pad = N_CORES * NPC - n
    if pad:
        qpad = np.tile(np.array([1, 0, 0, 0], np.float32), (pad, 1))
        lpad = np.zeros((pad, 3), np.float32)
        quaternion = np.concatenate([quaternion, qpad], axis=0)
        log_scale = np.concatenate([log_scale, lpad], axis=0)
    in_maps = []
    for i in range(N_CORES):
        sl = slice(i * NPC, (i + 1) * NPC)
        in_maps.append({
            "q": np.ascontiguousarray(quaternion[sl]),
            "ls": np.ascontiguousarray(log_scale[sl]),
        })
    return in_maps


def kernel_with_stats(quaternion, log_scale, trace=False):
    quaternion = np.asarray(quaternion, dtype=np.float32)
    log_scale = np.asarray(log_scale, dtype=np.float32)
    n = quaternion.shape[0]
    nc = _build()
    in_maps = _pad_and_shard(quaternion, log_scale)
    res = run_bass_kernel_spmd(nc, in_maps, core_ids=list(range(N_CORES)), trace=trace)
    full5 = np.concatenate([r["cov5"] for r in res.results], axis=1)[:, :n]
    c00 = full5[0].astype(np.float32)
    c11 = full5[1].astype(np.float32)
    trace_sig = np.exp(log_scale).sum(axis=1)       # = c00+c11+c22 exactly
    out = np.empty((n, 9), dtype=np.float32)
    out[:, 0] = c00
    out[:, 4] = c11
    out[:, 8] = trace_sig - c00 - c11
    out[:, 1] = out[:, 3] = full5[2]
    out[:, 5] = out[:, 7] = full5[3]
    out[:, 2] = out[:, 6] = full5[4]
    return out.reshape(n, 3, 3), res


def kernel(quaternion, log_scale):
    out, _ = kernel_with_stats(quaternion, log_scale, trace=False)
    return out


# revision 10
# speedup vs baseline: 1.0538x; 1.0538x over previous
"""Trainium2 Bass kernel: per-point 3x3 Gaussian covariance from quaternion + log_scale.

cov = R diag(exp(log_scale)) R^T with R from the normalized quaternion.

v3c: grouped [P, 3f] fp16 tiles with shifted views; deferred normalization
(raw quaternion products; 1/|q|^2 folded into per-column scales); diagonal
from the |q|^2 squares (u-trick); Gram via ScalarE squares + rotated cross
products; cov22 rebuilt on host from trace(cov) = sum(exp(log_scale));
planar 5-entry fp16 output, host assembles [N,3,3] fp32.
ScalarE work is software-pipelined one tile ahead (deinterleave/exp of tile
t+1 issue before the squares of tile t) so VectorE never waits at tile
boundaries.
"""

import os
import numpy as np

import concourse.bass as bass
import concourse.bacc as bacc
import concourse.mybir as mybir
from concourse.tile import TileContext
from concourse.bass_utils import run_bass_kernel_spmd

AF = mybir.ActivationFunctionType
ALU = mybir.AluOpType
FP32 = mybir.dt.float32
F16 = mybir.dt.float16

N_CORES = 8
N_FULL = 4_000_000
P = 128
R = 3908
NPC = P * R
F = int(os.environ.get("KERNEL_F", "782"))
SQ_ENG = os.environ.get("KERNEL_SQ_ENG", "s")

_built = {}


def _tiles():
    out, t0 = [], 0
    while t0 < R:
        f = min(F, R - t0)
        out.append((t0, f))
        t0 += f
    return out


def _build():
    key = (F, SQ_ENG)
    if key in _built:
        return _built[key]

    nc = bacc.Bacc("TRN2", target_bir_lowering=False, debug=False, num_devices=N_CORES)
    q = nc.dram_tensor("q", [NPC, 4], FP32, kind="ExternalInput")
    ls = nc.dram_tensor("ls", [NPC, 3], FP32, kind="ExternalInput")
    cov5 = nc.dram_tensor("cov5", [5, NPC], F16, kind="ExternalOutput")

    qv = q.ap().rearrange("(p r) c -> p (r c)", p=P)
    lsv = ls.ap().rearrange("(p r) c -> p (r c)", p=P)
    ov = cov5.ap().rearrange("e (p r) -> p e r", p=P)

    with TileContext(nc) as tc:
        with (
            tc.tile_pool(name="io", bufs=2) as io,
            tc.tile_pool(name="wk", bufs=1) as wk,
        ):
            tiles = _tiles()
            phase1 = [None] * len(tiles)
            phase1[0] = _load_phase(nc, io, qv, lsv, *tiles[0])
            for i, (t0, f) in enumerate(tiles):
                nxt = i + 1
                if nxt < len(tiles):
                    def load_next():
                        phase1[nxt] = _load_phase(nc, io, qv, lsv, *tiles[nxt])
                else:
                    load_next = lambda: None
                _tile_body(nc, io, wk, ov, t0, f, phase1[i], load_next)

    nc.compile()
    _built[key] = nc
    return nc


def _load_phase(nc, io, qv, lsv, t0, f):
    """DMA in + ScalarE deinterleave/exp for one tile."""
    def t2(n_f, dt, tag):
        return io.tile([P, n_f * f], dt, tag=tag, name=f"{tag}_{t0}")

    qt = t2(4, FP32, "qt")
    lst = t2(3, FP32, "lst")
    nc.sync.dma_start(out=qt, in_=qv[:, 4 * t0:4 * (t0 + f)])
    nc.sync.dma_start(out=lst, in_=lsv[:, 3 * t0:3 * (t0 + f)])
    qc = qt.rearrange("p (f c) -> p f c", c=4)
    lsc = lst.rearrange("p (f c) -> p f c", c=3)

    Q4 = t2(4, F16, "Q4")                                  # [w|x|y|z]
    for c in range(4):
        nc.scalar.copy(out=Q4[:, c * f:(c + 1) * f], in_=qc[:, :, c])
    ER3 = t2(3, F16, "ER3")                                # [e0|e1|e2]
    for j in range(3):
        nc.scalar.activation(ER3[:, j * f:(j + 1) * f], lsc[:, :, j], AF.Exp,
                             scale=0.5)
    return Q4, ER3


def _tile_body(nc, io, wk, ov, t0, f, ph1, load_next):
    Q4, ER3 = ph1

    def t2(n_f, dt, tag):
        return io.tile([P, n_f * f], dt, tag=tag, name=f"{tag}_{t0}")

    def t1(n_f, tag, dt=F16):
        return wk.tile([P, n_f * f], dt, tag=tag, name=f"{tag}_{t0}")

    # ---------------- VectorE: squares -> n2, diag entries -----------------
    q2 = t1(4, "q2")
    nc.vector.tensor_mul(q2, Q4, Q4)                       # ww|xx|yy|zz
    u1 = t1(1, "u1"); u2 = t1(1, "u2"); u3 = t1(1, "u3"); u4 = t1(1, "u4")
    nc.vector.tensor_add(u1, q2[:, :f], q2[:, f:2 * f])            # ww+xx
    nc.vector.tensor_add(u2, q2[:, 2 * f:3 * f], q2[:, 3 * f:])    # yy+zz
    n2 = t2(1, F16, "n2")
    nc.vector.tensor_add(n2, u1, u2)
    nc.vector.tensor_sub(u3, q2[:, :f], q2[:, f:2 * f])            # ww-xx
    nc.vector.tensor_sub(u4, q2[:, 2 * f:3 * f], q2[:, 3 * f:])    # yy-zz
    rdr = t1(3, "rdr")                                     # n2*(r00,r11,r22)
    nc.vector.tensor_sub(rdr[:, :f], u1, u2)
    nc.vector.tensor_add(rdr[:, f:2 * f], u3, u4)
    nc.vector.tensor_sub(rdr[:, 2 * f:], u3, u4)

    # ---------------- ScalarE: 1/n2 + Q4b permute --------------------------
    L = t2(1, F16, "L")
    nc.scalar.activation(L, n2, AF.Ln)
    sh = t2(1, F16, "sh")                                  # 1/n2
    nc.scalar.activation(sh, L, AF.Exp, scale=-1.0)
    Q4b = t2(3, F16, "Q4b")                                # [z|x|y]
    nc.scalar.copy(out=Q4b[:, :f], in_=Q4[:, 3 * f:])
    nc.scalar.copy(out=Q4b[:, f:], in_=Q4[:, f:3 * f])

    def bcast3(ap_1f):
        return ap_1f.rearrange("p (o f) -> p o f", o=1).to_broadcast([P, 3, f])

    def g3(ap_3f):
        return ap_3f.rearrange("p (o f) -> p o f", o=3)

    # ---------------- VectorE: off-diagonal raw products -------------------
    Um = t1(3, "Um")                                       # (xy, yz, xz)
    nc.vector.tensor_mul(Um[:, :2 * f], Q4[:, f:3 * f], Q4[:, 2 * f:])
    nc.vector.tensor_mul(Um[:, 2 * f:], Q4[:, f:2 * f], Q4[:, 3 * f:])
    Vm = t1(3, "Vm")                                       # (wz, wx, wy)
    nc.vector.tensor_mul(g3(Vm), bcast3(Q4[:, :f]), g3(Q4b))
    rm = t1(3, "rm")                                       # (xy-wz, yz-wx, xz-wy)
    nc.vector.tensor_sub(rm, Um, Vm)
    padd4 = t1(4, "padd4")                                 # [p02|p10|p21|p02]
    nc.vector.tensor_add(padd4[:, f:], Um, Vm)             # (p10,p21,p02)
    nc.vector.tensor_copy(out=padd4[:, :f], in_=padd4[:, 3 * f:])

    # ---------------- column scales ----------------------------------------
    # rdr = n2*R_ii ; rm/padd = n2*R_ij/2 -> diag scale e_j/n2, off 2*e_j/n2
    Escd = t1(3, "Escd")
    nc.vector.tensor_mul(g3(Escd), g3(ER3), bcast3(sh))    # e_j/n2
    Esco5 = t1(5, "Esco5")
    nc.vector.tensor_scalar(Esco5[:, :3 * f], Escd, 2.0, None, ALU.mult)
    nc.vector.tensor_copy(out=Esco5[:, 3 * f:], in_=Esco5[:, :2 * f])

    # ---------------- M rows ----------------------------------------------
    T0e = t1(4, "T0e"); T1e = t1(4, "T1e"); T2e = t1(4, "T2e")
    nc.vector.tensor_mul(T0e[:, :3 * f], rdr, Escd)                # (M00,M11,M22)
    nc.vector.tensor_mul(T1e[:, :3 * f], rm, Esco5[:, f:4 * f])    # (M01,M12,M20)
    nc.vector.tensor_mul(T2e[:, :3 * f], padd4[:, :3 * f], Esco5[:, 2 * f:])  # (M02,M10,M21)
    for Te in (T0e, T1e, T2e):
        nc.vector.tensor_copy(out=Te[:, 3 * f:], in_=Te[:, :f])

    # next tile's DMA + ScalarE phase-1 issue here, ahead of the squares,
    # so ScalarE keeps VectorE fed across the tile boundary
    load_next()

    # ---------------- Gram -------------------------------------------------
    S0 = t1(2, "S0"); S1 = t1(2, "S1"); S2 = t1(2, "S2")
    if SQ_ENG == "s":
        nc.scalar.activation(S0, T0e[:, :2 * f], AF.Square)
        nc.scalar.activation(S1, T1e[:, :2 * f], AF.Square)
        nc.scalar.activation(S2, T2e[:, :2 * f], AF.Square)
    else:
        nc.vector.tensor_mul(S0, T0e[:, :2 * f], T0e[:, :2 * f])
        nc.vector.tensor_mul(S1, T1e[:, :2 * f], T1e[:, :2 * f])
        nc.vector.tensor_mul(S2, T2e[:, :2 * f], T2e[:, :2 * f])
    X = t1(3, "X"); Y = t1(3, "Y"); Z = t1(3, "Z")
    nc.vector.tensor_mul(X, T0e[:, :3 * f], T2e[:, f:])
    nc.vector.tensor_mul(Y, T1e[:, :3 * f], T0e[:, f:])
    nc.vector.tensor_mul(Z, T2e[:, :3 * f], T1e[:, f:])

    ot = t2(5, F16, "ot")                 # [c00|c11|c01|c12|c02]
    dsum = t1(2, "dsum"); osum = t1(3, "osum")
    nc.vector.tensor_add(dsum, S0, S1)
    nc.vector.tensor_add(ot[:, :2 * f], dsum, S2)
    nc.vector.tensor_add(osum, X, Y)
    nc.vector.tensor_add(ot[:, 2 * f:], osum, Z)

    otv = ot.rearrange("p (e f) -> p e f", e=5)
    nc.sync.dma_start(out=ov[:, :, t0:t0 + f], in_=otv)


def _pad_and_shard(quaternion, log_scale):
    n = quaternion.shape[0]
    pad = N_CORES * NPC - n
    if pad:
        qpad = np.tile(np.array([1, 0, 0, 0], np.float32), (pad, 1))
        lpad = np.zeros((pad, 3), np.float32)
        quaternion = np.concatenate([quaternion, qpad], axis=0)
        log_scale = np.concatenate([log_scale, lpad], axis=0)
    in_maps = []
    for i in range(N_CORES):
        sl = slice(i * NPC, (i + 1) * NPC)
        in_maps.append({
            "q": np.ascontiguousarray(quaternion[sl]),
            "ls": np.ascontiguousarray(log_scale[sl]),
        })
    return in_maps


def kernel_with_stats(quaternion, log_scale, trace=False):
    quaternion = np.asarray(quaternion, dtype=np.float32)
    log_scale = np.asarray(log_scale, dtype=np.float32)
    n = quaternion.shape[0]
    nc = _build()
    in_maps = _pad_and_shard(quaternion, log_scale)
    res = run_bass_kernel_spmd(nc, in_maps, core_ids=list(range(N_CORES)), trace=trace)
    full5 = np.concatenate([r["cov5"] for r in res.results], axis=1)[:, :n]
    c00 = full5[0].astype(np.float32)
    c11 = full5[1].astype(np.float32)
    trace_sig = np.exp(log_scale).sum(axis=1)       # = c00+c11+c22 exactly
    out = np.empty((n, 9), dtype=np.float32)
    out[:, 0] = c00
    out[:, 4] = c11
    out[:, 8] = trace_sig - c00 - c11
    out[:, 1] = out[:, 3] = full5[2]
    out[:, 5] = out[:, 7] = full5[3]
    out[:, 2] = out[:, 6] = full5[4]
    return out.reshape(n, 3, 3), res


def kernel(quaternion, log_scale):
    out, _ = kernel_with_stats(quaternion, log_scale, trace=False)
    return out
